# revision 1
# baseline (speedup 1.0000x reference)
"""Trainium2 Bass kernel for nn_BasicDNC (4-layer transformer + external
memory read + tied LM head), SPMD over 8 NeuronCores.

Sharding:
  - tokens (B*T = 4096) split 512/core; cores 0-3 own batch 0, 4-7 batch 1
  - attention K/V allgathered within each 4-core batch group
  - memory bank + lm head token-sharded (full mem_K^T / tok_embed^T per core)

Layouts: activations kept transposed ([d, tok]) so every matmul contracts
over the partition dim; scores / logits come out in [tok, free] layout.
"""
import sys

sys.path.insert(0, "/opt/trn_rl_repo")

import numpy as np
import ml_dtypes

import concourse.bass as bass
import concourse.bacc as bacc
import concourse.mybir as mybir
import concourse.tile as tile
from concourse.bass_utils import run_bass_kernel_spmd
from concourse.masks import make_identity

F32 = mybir.dt.float32
BF16 = mybir.dt.bfloat16
FP16 = mybir.dt.float16
U8 = mybir.dt.uint8
U16 = mybir.dt.uint16
U32 = mybir.dt.uint32
I32 = mybir.dt.int32
AF = mybir.ActivationFunctionType
ALU = mybir.AluOpType
AX = mybir.AxisListType

N_CORES = 8
P = 128
D = 512
H = 8
DH = 64
L = 4
FF = 2048
B = 2
T = 2048
TOK = 512          # tokens per core
NT = TOK // P      # token tiles per core (4)
ND = D // P        # d tiles (4)
NFF = FF // P      # ff tiles (16)
NK = T // P        # key tiles per batch group (16)
SLOTS = 32768
SC = SLOTS // 2    # max-op chunk (16384)
V_SIZE = 32000
TOPK = 8
EPS = 1e-8
VSW = 65           # per-head v-store width (64 v cols + 1 ones col)
SSH = SLOTS // N_CORES   # slots per core (4096)
VSH = V_SIZE // N_CORES  # vocab per core (4000)


def build_program(flags):
    nc = bacc.Bacc(None, num_devices=N_CORES)

    # ---------------- io ----------------
    x0T_d = nc.dram_tensor("x0T", [D, TOK], F32, kind="ExternalInput")
    posT_d = nc.dram_tensor("posT", [D, TOK], F32, kind="ExternalInput")
    wq_d = nc.dram_tensor("wq", [L, D, D], BF16, kind="ExternalInput")
    wk_d = nc.dram_tensor("wk", [L, D, D], BF16, kind="ExternalInput")
    wv_d = nc.dram_tensor("wv", [L, D, D], BF16, kind="ExternalInput")
    wo_d = nc.dram_tensor("wo", [L, D, D], BF16, kind="ExternalInput")
    w1_d = nc.dram_tensor("w1", [L, D, FF], BF16, kind="ExternalInput")
    w2_d = nc.dram_tensor("w2", [L, FF, D], BF16, kind="ExternalInput")
    wqm_d = nc.dram_tensor("wqm", [D, D], BF16, kind="ExternalInput")
    wr_d = nc.dram_tensor("wr", [D, D], BF16, kind="ExternalInput")
    mkT_d = nc.dram_tensor("mkT", [D, SSH], BF16, kind="ExternalInput")
    mv_d = nc.dram_tensor("mv", [SLOTS, D], BF16, kind="ExternalInput")
    eT_d = nc.dram_tensor("eT", [D, VSH], BF16, kind="ExternalInput")
    if flags["bias"]:
        bo_d = nc.dram_tensor("bo", [L, D], F32, kind="ExternalInput")
        b1_d = nc.dram_tensor("b1", [L, FF], F32, kind="ExternalInput")
        b2_d = nc.dram_tensor("b2", [L, D], F32, kind="ExternalInput")
        bqm_d = nc.dram_tensor("bqm", [D], F32, kind="ExternalInput")
        br_d = nc.dram_tensor("br", [D], F32, kind="ExternalInput")
    if flags["normw"]:
        n1_d = nc.dram_tensor("n1", [L, D], F32, kind="ExternalInput")
        n2_d = nc.dram_tensor("n2", [L, D], F32, kind="ExternalInput")
        no_d = nc.dram_tensor("no", [D], F32, kind="ExternalInput")
    if flags["salience"]:
        sal_d = nc.dram_tensor("sal", [1, SSH], F32, kind="ExternalInput")

    logits_d = nc.dram_tensor("logits", [B * T, VSH], FP16, kind="ExternalOutput")
    debug = flags.get("debug", False)
    if debug:
        dbg_emb = nc.dram_tensor("dbg_emb", [D, TOK], F32, kind="ExternalOutput")
        dbg_lyr = nc.dram_tensor("dbg_lyr", [L, D, TOK], F32, kind="ExternalOutput")
        dbg_mqT = nc.dram_tensor("dbg_mqT", [D, TOK], F32, kind="ExternalOutput")
        dbg_v8 = nc.dram_tensor("dbg_v8", [TOK, 8], F32, kind="ExternalOutput")
        dbg_idx = nc.dram_tensor("dbg_idx", [TOK, 8], U32, kind="ExternalOutput")
        dbg_read = nc.dram_tensor("dbg_read", [TOK, D], F32, kind="ExternalOutput")
        dbg_xout = nc.dram_tensor("dbg_xout", [D, TOK], F32, kind="ExternalOutput")
        dbg_kf = nc.dram_tensor("dbg_kf", [D, 4 * TOK], F32, kind="ExternalOutput")
        dbg_ao = nc.dram_tensor("dbg_ao", [H * DH, TOK], F32, kind="ExternalOutput")
        dbg_vf = nc.dram_tensor("dbg_vf", [4 * TOK, H * VSW], F32, kind="ExternalOutput")
        dbg_den = nc.dram_tensor("dbg_den", [H, TOK], F32, kind="ExternalOutput")
        dbg_pex = nc.dram_tensor("dbg_pex", [P, TOK], F32, kind="ExternalOutput")

    groups = [[0, 1, 2, 3], [4, 5, 6, 7]]
    ablate = flags.get("ablate", frozenset())

    import contextlib

    with tile.TileContext(nc) as tc, contextlib.ExitStack() as ctx:
        persist = ctx.enter_context(tc.tile_pool(name="persist", bufs=1))
        consts = ctx.enter_context(tc.tile_pool(name="consts", bufs=1))
        dram = ctx.enter_context(tc.tile_pool(name="dram", bufs=2, space="DRAM"))
        psum_s = ctx.enter_context(tc.tile_pool(name="psum_s", bufs=4, space="PSUM"))
        psum_a = ctx.enter_context(tc.tile_pool(name="psum_a", bufs=1, space="PSUM"))
        small = ctx.enter_context(tc.tile_pool(name="small", bufs=2))

        ones_bf = consts.tile([P, 1], BF16, tag="ones_bf", name="ones_bf")
        nc.vector.memset(ones_bf[:], 1.0)
        eps_t = consts.tile([1, 1], F32, tag="eps_t", name="eps_t")
        nc.vector.memset(eps_t[:], EPS)

        # persistent residual stream (transposed): xT fp32 + bf16 shadow
        xT = [persist.tile([P, TOK], F32, tag=f"xT{t}", name=f"xT{t}") for t in range(ND)]
        xq = [persist.tile([P, TOK], BF16, tag=f"xq{t}", name=f"xq{t}") for t in range(ND)]

        def rms_norm(tiles_f32, tiles_bf16, normw_ap):
            """In-place RMS norm over d (partition dim x ND tiles) of the
            [d, tok] stream; refresh bf16 shadow."""
            ssq = psum_a.tile([1, TOK], F32, tag="ssq", name="ssq")
            for t in range(ND):
                sq = small.tile([P, TOK], BF16, tag="sq", name="sq")
                nc.scalar.activation(sq[:], tiles_f32[t][:], AF.Square)
                nc.tensor.matmul(ssq[:], lhsT=ones_bf[:], rhs=sq[:],
                                 start=(t == 0), stop=(t == ND - 1))
            scale1 = small.tile([1, TOK], F32, tag="scale1", name="scale1")
            nc.scalar.activation(scale1[:], ssq[:], AF.Sqrt,
                                 bias=eps_t[:, 0:1], scale=1.0 / D)
            scaleb = small.tile([P, TOK], F32, tag="scaleb", name="scaleb")
            nc.gpsimd.partition_broadcast(scaleb[:], scale1[:])
            nc.vector.reciprocal(scaleb[:], scaleb[:])
            for t in range(ND):
                nc.vector.tensor_mul(tiles_f32[t][:], tiles_f32[t][:], scaleb[:])
                if normw_ap is not None:
                    nw = small.tile([P, 1], F32, tag="nw", name="nw")
                    nc.sync.dma_start(nw[:], normw_ap[t * P:(t + 1) * P, None])
                    nc.vector.tensor_scalar(tiles_f32[t][:], tiles_f32[t][:],
                                            nw[:, 0:1], scalar2=None, op0=ALU.mult)
                nc.scalar.activation(tiles_bf16[t][:], tiles_f32[t][:], AF.Copy)

        def load_bias_tile(pool, ap_1d, t, tag):
            bt = pool.tile([P, 1], F32, tag=tag)
            nc.sync.dma_start(bt[:], ap_1d[t * P:(t + 1) * P, None])
            return bt

        # ---------------- embedding ----------------
        for t in range(ND):
            p0 = small.tile([P, TOK], F32, tag="emb0", name="emb0")
            p1 = small.tile([P, TOK], F32, tag="emb1", name="emb1")
            nc.sync.dma_start(p0[:], x0T_d[t * P:(t + 1) * P, :])
            nc.sync.dma_start(p1[:], posT_d[t * P:(t + 1) * P, :])
            nc.vector.tensor_add(xT[t][:], p0[:], p1[:])
            nc.scalar.activation(xq[t][:], xT[t][:], AF.Copy)
            if debug:
                nc.sync.dma_start(dbg_emb[t * P:(t + 1) * P, :], xT[t][:])

        # ---------------- transformer layers ----------------
        n_layers = 0 if "layers" in ablate else L
        with (
            tc.tile_pool(name="wts", bufs=1) as wts,
            tc.tile_pool(name="attn", bufs=1) as attn,
            tc.tile_pool(name="psb", bufs=2) as psb,
            tc.tile_pool(name="ffh", bufs=1) as ffh,
        ):
            for l in range(n_layers):
                # --- QKV projections ---
                wq_sb = [wts.tile([P, D], BF16, tag=f"wq{c}", name=f"wq{c}") for c in range(ND)]
                wk_sb = [wts.tile([P, D], BF16, tag=f"wk{c}", name=f"wk{c}") for c in range(ND)]
                wv_sb = [wts.tile([P, D], BF16, tag=f"wv{c}", name=f"wv{c}") for c in range(ND)]
                wo_sb = [wts.tile([DH, D], BF16, tag=f"wo{c}", name=f"wo{c}") for c in range(H)]
                for c in range(ND):
                    nc.sync.dma_start(wq_sb[c][:], wq_d[l, c * P:(c + 1) * P, :])
                    nc.sync.dma_start(wk_sb[c][:], wk_d[l, c * P:(c + 1) * P, :])
                    nc.sync.dma_start(wv_sb[c][:], wv_d[l, c * P:(c + 1) * P, :])
                for c in range(H):
                    nc.sync.dma_start(wo_sb[c][:], wo_d[l, c * DH:(c + 1) * DH, :])

                qT = [attn.tile([P, TOK], BF16, tag=f"qT{m}", name=f"qT{m}") for m in range(ND)]
                kT = [attn.tile([P, TOK], BF16, tag=f"kT{m}", name=f"kT{m}") for m in range(ND)]
                vst = [attn.tile([P, H, VSW], BF16, tag=f"vst{m}", name=f"vst{m}") for m in range(NT)]

                ag_in = dram.tile([D, TOK + H * VSW], BF16, tag="ag_in", name="ag_in")
                ag_out = dram.tile([4 * D, TOK + H * VSW], BF16, tag="ag_out", name="ag_out")

                for m in range(ND):
                    ps = psum_s.tile([P, TOK], F32, tag="ps", name="ps")
                    for c in range(ND):
                        nc.tensor.matmul(ps[:], lhsT=wq_sb[c][:, m * P:(m + 1) * P],
                                         rhs=xq[c][:], start=(c == 0), stop=(c == ND - 1))
                    nc.scalar.activation(qT[m][:], ps[:], AF.Copy)
                    ps = psum_s.tile([P, TOK], F32, tag="ps", name="ps")
                    for c in range(ND):
                        nc.tensor.matmul(ps[:], lhsT=wk_sb[c][:, m * P:(m + 1) * P],
                                         rhs=xq[c][:], start=(c == 0), stop=(c == ND - 1))
                    nc.scalar.activation(kT[m][:], ps[:], AF.Copy)
                    nc.sync.dma_start(ag_in[m * P:(m + 1) * P, 0:TOK], kT[m][:])
                for m in range(NT):
                    ps = psum_s.tile([P, TOK], F32, tag="ps", name="ps")
                    for c in range(ND):
                        nc.tensor.matmul(ps[:], lhsT=xq[c][:, m * P:(m + 1) * P],
                                         rhs=wv_sb[c][:], start=(c == 0), stop=(c == ND - 1))
                    pv = ps[:].rearrange("p (h d) -> p h d", h=H)
                    nc.vector.tensor_copy(vst[m][:, :, 0:DH], pv[:])
                    nc.vector.memset(vst[m][:, :, DH:VSW], 1.0)
                    nc.sync.dma_start(
                        ag_in[m * P:(m + 1) * P, TOK:].rearrange(
                            "p (h w) -> p h w", h=H),
                        vst[m][:])

                nc.gpsimd.collective_compute(
                    "AllGather", ALU.bypass, replica_groups=groups,
                    ins=[ag_in.opt()], outs=[ag_out.opt()])

                ago = ag_out[:].rearrange("(r x) c -> r x c", r=4)
                kfull = [attn.tile([P, 4 * TOK], BF16, tag=f"kfull{t}", name=f"kfull{t}")
                         for t in range(ND)]
                for t in range(ND):
                    nc.sync.dma_start(
                        kfull[t][:].rearrange("p (r t) -> p r t", r=4),
                        ago[:, t * P:(t + 1) * P, 0:TOK].rearrange("r p t -> p r t"))
                vfull = [attn.tile([P, H, VSW], BF16, tag=f"vfull{t}", name=f"vfull{t}")
                         for t in range(NK)]
                for t in range(NK):
                    r, rr = t // NT, t % NT
                    nc.sync.dma_start(
                        vfull[t][:],
                        ago[r, rr * P:(rr + 1) * P, TOK:].rearrange(
                            "p (h w) -> p h w", h=H))

                if debug and l == 0:
                    for t in range(ND):
                        nc.gpsimd.dma_start(dbg_kf[t * P:(t + 1) * P, :], kfull[t][:])
                    for t in range(NK):
                        nc.gpsimd.dma_start(
                            dbg_vf[t * P:(t + 1) * P, :],
                            vfull[t][:].rearrange("p h w -> p (h w)"))

                # --- attention (scores kept transposed [k, q]) ---
                # per-head AO in [64, TOK] tiles at base partition 0; the
                # O-projection contracts per-head (C=64) so no restacking.
                ao_h = [attn.tile([DH, TOK], BF16, tag=f"ao_h{h}", name=f"ao_h{h}")
                        for h in range(H)]
                for h in range(H):
                    dt, lo = h // 2, (h % 2) * DH
                    pall = psb.tile([P, NK, TOK], BF16, tag="pexp", name="pexp")
                    for kt in range(NK):
                        ps = psum_s.tile([P, TOK], F32, tag="ps", name="ps")
                        nc.tensor.matmul(
                            ps[:],
                            lhsT=kfull[dt][lo:lo + DH, kt * P:(kt + 1) * P],
                            rhs=qT[dt][lo:lo + DH, :], start=True, stop=True)
                        nc.scalar.activation(pall[:, kt, :], ps[:], AF.Exp,
                                             scale=DH ** -0.5)
                        if debug and l == 0 and h == 0 and kt == 0:
                            nc.gpsimd.dma_start(dbg_pex[:], pall[:, 0, :])
                    aops = psum_a.tile([P, TOK], F32, tag=f"ao{h % 2}", name=f"ao{h % 2}")
                    for kt in range(NK):
                        nc.tensor.matmul(aops[0:VSW, :], lhsT=vfull[kt][:, h, :],
                                         rhs=pall[:, kt, :],
                                         start=(kt == 0), stop=(kt == NK - 1))
                    # per-head denominator: row DH -> bcast -> recip -> normalize
                    den1 = attn.tile([P, TOK], F32, tag="den1", name="den1")
                    nc.scalar.activation(den1[DH:DH + 1, :], aops[DH:DH + 1, :],
                                         AF.Copy)
                    den0 = attn.tile([1, TOK], F32, tag="den0", name="den0")
                    nc.sync.dma_start(den0[:], den1[DH:DH + 1, :])
                    denb = attn.tile([P, TOK], F32, tag="denb", name="denb")
                    nc.gpsimd.partition_broadcast(denb[:], den0[:])
                    if debug and l == 0:
                        nc.gpsimd.dma_start(dbg_den[h:h + 1, :], den1[DH:DH + 1, :])
                    nc.vector.reciprocal(denb[0:DH, :], denb[0:DH, :])
                    nc.vector.tensor_mul(ao_h[h][:], aops[0:DH, :], denb[0:DH, :])
                    if debug and l == 0:
                        nc.gpsimd.dma_start(dbg_ao[h * DH:(h + 1) * DH, :], ao_h[h][:])

                # --- output projection + residual + norm1 ---
                for m in range(ND):
                    ps = psum_s.tile([P, TOK], F32, tag="ps", name="ps")
                    for h in range(H):
                        nc.tensor.matmul(
                            ps[:], lhsT=wo_sb[h][:, m * P:(m + 1) * P],
                            rhs=ao_h[h][:], start=(h == 0), stop=(h == H - 1))
                    nc.vector.tensor_add(xT[m][:], xT[m][:], ps[:])
                    if flags["bias"]:
                        bt = load_bias_tile(small, bo_d[l], m, "bo")
                        nc.vector.tensor_scalar(xT[m][:], xT[m][:], bt[:, 0:1],
                                                scalar2=None, op0=ALU.add)
                rms_norm(xT, xq, n1_d[l] if flags["normw"] else None)

                # --- FFN + residual + norm2 ---
                w1_sb = [wts.tile([P, FF], BF16, tag=f"w1_{c}", name=f"w1_{c}") for c in range(ND)]
                w2_sb = [wts.tile([P, D], BF16, tag=f"w2_{c}", name=f"w2_{c}") for c in range(NFF)]
                for c in range(ND):
                    nc.sync.dma_start(w1_sb[c][:], w1_d[l, c * P:(c + 1) * P, :])
                for c in range(NFF):
                    nc.sync.dma_start(w2_sb[c][:], w2_d[l, c * P:(c + 1) * P, :])
                hT = [ffh.tile([P, TOK], BF16, tag=f"hT{f}", name=f"hT{f}") for f in range(NFF)]
                for f in range(NFF):
                    ps = psum_s.tile([P, TOK], F32, tag="ps", name="ps")
                    for c in range(ND):
                        nc.tensor.matmul(ps[:], lhsT=w1_sb[c][:, f * P:(f + 1) * P],
                                         rhs=xq[c][:], start=(c == 0), stop=(c == ND - 1))
                    if flags["bias"]:
                        bt = load_bias_tile(small, b1_d[l], f, "b1")
                        nc.scalar.activation(hT[f][:], ps[:], AF.Gelu_apprx_tanh,
                                             bias=bt[:, 0:1])
                    else:
                        nc.scalar.activation(hT[f][:], ps[:], AF.Gelu_apprx_tanh)
                for m in range(ND):
                    ps = psum_s.tile([P, TOK], F32, tag="ps", name="ps")
                    for c in range(NFF):
                        nc.tensor.matmul(ps[:], lhsT=w2_sb[c][:, m * P:(m + 1) * P],
                                         rhs=hT[c][:], start=(c == 0), stop=(c == NFF - 1))
                    nc.vector.tensor_add(xT[m][:], xT[m][:], ps[:])
                    if flags["bias"]:
                        bt = load_bias_tile(small, b2_d[l], m, "b2")
                        nc.vector.tensor_scalar(xT[m][:], xT[m][:], bt[:, 0:1],
                                                scalar2=None, op0=ALU.add)
                rms_norm(xT, xq, n2_d[l] if flags["normw"] else None)
                if debug:
                    for t in range(ND):
                        nc.sync.dma_start(dbg_lyr[l, t * P:(t + 1) * P, :], xT[t][:])

        # ---------------- external memory read ----------------
        xoutq = [persist.tile([P, TOK], BF16, tag=f"xoq{t}", name=f"xoq{t}") for t in range(ND)]
        if "mem" in ablate:
            for t in range(ND):
                nc.vector.tensor_copy(xoutq[t][:], xq[t][:])
        if "mem" not in ablate:
         with (
            tc.tile_pool(name="mem", bufs=1) as mem,
            tc.tile_pool(name="scpool", bufs=1) as scpool,
            tc.tile_pool(name="kch", bufs=2) as kch,
            tc.tile_pool(name="msmall", bufs=2) as msmall,
            tc.tile_pool(name="gat", bufs=1) as gat,
        ):
            # mq^T = wqm^T x  (scaled by D^-0.5 on copy)
            wqm_sb = [mem.tile([P, D], BF16, tag=f"wqm{c}", name=f"wqm{c}") for c in range(ND)]
            for c in range(ND):
                nc.sync.dma_start(wqm_sb[c][:], wqm_d[c * P:(c + 1) * P, :])
            mqT = [mem.tile([P, TOK], BF16, tag=f"mqT{m}", name=f"mqT{m}") for m in range(ND)]
            for m in range(ND):
                ps = psum_s.tile([P, TOK], F32, tag="ps", name="ps")
                for c in range(ND):
                    nc.tensor.matmul(ps[:], lhsT=wqm_sb[c][:, m * P:(m + 1) * P],
                                     rhs=xq[c][:], start=(c == 0), stop=(c == ND - 1))
                if flags["bias"]:
                    bt = load_bias_tile(msmall, bqm_d, m, "bqm")
                    sc1 = msmall.tile([P, 1], F32, tag="bqms", name="bqms")
                    nc.vector.tensor_scalar(sc1[:], bt[:], float(D ** -0.5),
                                            scalar2=None, op0=ALU.mult)
                    nc.scalar.activation(mqT[m][:], ps[:], AF.Identity,
                                         bias=sc1[:, 0:1], scale=float(D ** -0.5))
                else:
                    nc.scalar.activation(mqT[m][:], ps[:], AF.Copy,
                                         scale=float(D ** -0.5))
                if debug:
                    nc.gpsimd.dma_start(dbg_mqT[m * P:(m + 1) * P, :], mqT[m][:])

            # --- allgather mq over all 8 cores (slot-sharded scoring) ---
            NTT = (B * T) // P          # 32 token tiles over all tokens
            NSC = SSH // TOK            # 8 slot chunks of 512
            mq_in = dram.tile([D, TOK], BF16, tag="mq_in", name="mq_in")
            mq_out = dram.tile([N_CORES * D, TOK], BF16, tag="mq_out", name="mq_out")
            for m in range(ND):
                nc.sync.dma_start(mq_in[m * P:(m + 1) * P, :], mqT[m][:])
            nc.gpsimd.collective_compute(
                "AllGather", ALU.bypass, replica_groups=[list(range(N_CORES))],
                ins=[mq_in.opt()], outs=[mq_out.opt()])
            mqo = mq_out[:].rearrange("(r x) c -> r x c", r=N_CORES)
            mqall = [mem.tile([P, B * T], BF16, tag=f"mqa{c}", name=f"mqa{c}")
                     for c in range(ND)]
            for c in range(ND):
                nc.sync.dma_start(
                    mqall[c][:].rearrange("p (r t) -> p r t", r=N_CORES),
                    mqo[:, c * P:(c + 1) * P, :].rearrange("r p t -> p r t"))

            # resident mem_K^T shard [D, SSH]
            mk_sb = [mem.tile([P, SSH], BF16, tag=f"mk{c}", name=f"mk{c}")
                     for c in range(ND)]
            for c in range(ND):
                nc.sync.dma_start(mk_sb[c][:], mkT_d[c * P:(c + 1) * P, :])

            if flags["salience"]:
                salb = mem.tile([P, SSH], BF16, tag="salb", name="salb")
                sal_sb = mem.tile([1, SSH], F32, tag="sal1", name="sal1")
                nc.sync.dma_start(sal_sb[:], sal_d[:])
                nc.gpsimd.partition_broadcast(salb[:], sal_sb[:])

            iota64i = consts.tile([P, 64], I32, tag="iota_i", name="iota_i")
            nc.gpsimd.iota(iota64i[:], pattern=[[1, 64]], base=0,
                           channel_multiplier=0)
            iota64f = consts.tile([P, 64], F32, tag="iota_f", name="iota_f")
            nc.vector.tensor_copy(iota64f[:], iota64i[:])

            read_bf = [mem.tile([P, D], BF16, tag=f"read{t}", name=f"read{t}") for t in range(NT)]

            # --- score all tokens against my slot shard; local top-8 ---
            cval = mem.tile([P, NTT, 8], FP16, tag="cval", name="cval")
            cidx = mem.tile([P, NTT, 8], U16, tag="cidx", name="cidx")
            if "scores" in ablate:
                for t in range(NT):
                    nc.vector.memset(read_bf[t][:], 0.0)
            for tt in range(0 if "scores" in ablate else NTT):
                sct = scpool.tile([P, SSH], FP16, tag="sct", name="sct")
                for s in range(NSC):
                    ps = psum_s.tile([P, TOK], F32, tag="ps", name="ps")
                    for c in range(ND):
                        nc.tensor.matmul(ps[:],
                                         lhsT=mqall[c][:, tt * P:(tt + 1) * P],
                                         rhs=mk_sb[c][:, s * TOK:(s + 1) * TOK],
                                         start=(c == 0), stop=(c == ND - 1))
                    if flags["salience"]:
                        nc.vector.tensor_add(
                            sct[:, s * TOK:(s + 1) * TOK], ps[:],
                            salb[:, s * TOK:(s + 1) * TOK])
                    else:
                        nc.scalar.activation(
                            sct[:, s * TOK:(s + 1) * TOK], ps[:], AF.Copy)
                nc.vector.max(out=cval[:, tt, :], in_=sct[:])
                nc.vector.max_index(out=cidx[:, tt, :], in_max=cval[:, tt, :],
                                    in_values=sct[:])

            if "scores" not in ablate and "topk" not in ablate:
                # --- all-to-all: each core gets its own tokens' 64 candidates ---
                cv_in = dram.tile([B * T, 8], FP16, tag="cv_in", name="cv_in")
                ci_in = dram.tile([B * T, 8], U16, tag="ci_in", name="ci_in")
                cv_out = dram.tile([B * T, 8], FP16, tag="cv_out", name="cv_out")
                ci_out = dram.tile([B * T, 8], U16, tag="ci_out", name="ci_out")
                nc.sync.dma_start(
                    cv_in[:].rearrange("(tt p) k -> p tt k", p=P), cval[:])
                nc.sync.dma_start(
                    ci_in[:].rearrange("(tt p) k -> p tt k", p=P), cidx[:])
                nc.gpsimd.collective_compute(
                    "AllToAll", ALU.bypass, replica_groups=[list(range(N_CORES))],
                    ins=[cv_in.opt()], outs=[cv_out.opt()])
                nc.gpsimd.collective_compute(
                    "AllToAll", ALU.bypass, replica_groups=[list(range(N_CORES))],
                    ins=[ci_in.opt()], outs=[ci_out.opt()])
                cvo = cv_out[:].rearrange("(r lp) k -> r lp k", r=N_CORES)
                cio = ci_out[:].rearrange("(r lp) k -> r lp k", r=N_CORES)

                for t in range(NT):
                    v64 = msmall.tile([P, 64], FP16, tag="v64", name="v64")
                    i64 = msmall.tile([P, 64], U16, tag="i64", name="i64")
                    nc.sync.dma_start(
                        v64[:].rearrange("p (r k) -> p r k", r=N_CORES),
                        cvo[:, t * P:(t + 1) * P, :].rearrange("r p k -> p r k"))
                    nc.sync.dma_start(
                        i64[:].rearrange("p (r k) -> p r k", r=N_CORES),
                        cio[:, t * P:(t + 1) * P, :].rearrange("r p k -> p r k"))
                    # global candidate indices (+ r*SSH per rank block), f32
                    cidxf = msmall.tile([P, 64], F32, tag="cidxf", name="cidxf")
                    nc.vector.tensor_copy(cidxf[:], i64[:])
                    for r in range(1, N_CORES):
                        nc.vector.tensor_scalar(
                            cidxf[:, r * 8:(r + 1) * 8], cidxf[:, r * 8:(r + 1) * 8],
                            float(r * SSH), scalar2=None, op0=ALU.add)
                    v8 = msmall.tile([P, 8], FP16, tag="v8", name="v8")
                    nc.vector.max(out=v8[:], in_=v64[:])
                    pos = msmall.tile([P, 8], U16, tag="pos", name="pos")
                    nc.vector.max_index(out=pos[:], in_max=v8[:], in_values=v64[:])
                    posf = msmall.tile([P, 8], F32, tag="posf", name="posf")
                    nc.vector.tensor_copy(posf[:], pos[:])
                    eq = msmall.tile([P, 8, 64], F32, tag="eq", name="eq")
                    nc.vector.tensor_tensor(
                        out=eq[:], in0=posf[:, :, None].to_broadcast([P, 8, 64]),
                        in1=iota64f[:, None, :].to_broadcast([P, 8, 64]),
                        op=ALU.is_equal)
                    nc.vector.tensor_tensor(
                        out=eq[:], in0=eq[:],
                        in1=cidxf[:, None, :].to_broadcast([P, 8, 64]),
                        op=ALU.mult)
                    idxf = msmall.tile([P, 8], F32, tag="idxf", name="idxf")
                    nc.vector.reduce_sum(idxf[:], eq[:], axis=AX.X)
                    idxu = msmall.tile([P, 8], U32, tag="idxu", name="idxu")
                    nc.vector.tensor_copy(idxu[:], idxf[:])
                    # softmax over the 8 scores
                    vf = msmall.tile([P, 8], F32, tag="vf", name="vf")
                    nc.vector.tensor_copy(vf[:], v8[:])
                    ew = msmall.tile([P, 8], F32, tag="ew", name="ew")
                    nc.scalar.activation(ew[:], vf[:], AF.Exp)
                    ssum = msmall.tile([P, 1], F32, tag="ssum", name="ssum")
                    nc.vector.reduce_sum(ssum[:], ew[:], axis=AX.X)
                    rs = msmall.tile([P, 1], F32, tag="rs", name="rs")
                    nc.vector.reciprocal(rs[:], ssum[:])
                    w8 = msmall.tile([P, 8], F32, tag="w8", name="w8")
                    nc.vector.tensor_scalar(w8[:], ew[:], rs[:, 0:1],
                                            scalar2=None, op0=ALU.mult)
                    # gather mem_V rows and weighted-sum them
                    vsel = gat.tile([P, TOPK, D], BF16, tag="vsel", name="vsel")
                    for k in range(TOPK):
                        nc.gpsimd.indirect_dma_start(
                            out=vsel[:, k, :], out_offset=None, in_=mv_d[:],
                            in_offset=bass.IndirectOffsetOnAxis(
                                ap=idxu[:, k:k + 1], axis=0),
                            bounds_check=SLOTS - 1, oob_is_err=False)
                    racc = msmall.tile([P, D], F32, tag="racc", name="racc")
                    rtmp = msmall.tile([P, D], F32, tag="rtmp", name="rtmp")
                    nc.vector.tensor_scalar(racc[:], vsel[:, 0, :], w8[:, 0:1],
                                            scalar2=None, op0=ALU.mult)
                    for k in range(1, TOPK):
                        nc.vector.tensor_scalar(rtmp[:], vsel[:, k, :], w8[:, k:k + 1],
                                                scalar2=None, op0=ALU.mult)
                        nc.vector.tensor_add(racc[:], racc[:], rtmp[:])
                    nc.vector.tensor_copy(read_bf[t][:], racc[:])
                    if debug:
                        nc.sync.dma_start(dbg_read[t * P:(t + 1) * P, :], racc[:])
                        nc.sync.dma_start(dbg_idx[t * P:(t + 1) * P, :], idxu[:])
                        nc.gpsimd.dma_start(dbg_v8[t * P:(t + 1) * P, :], v8[:])
            elif "topk" in ablate:
                for t in range(NT):
                    nc.vector.memset(read_bf[t][:], 0.0)
                    nc.vector.tensor_scalar(
                        read_bf[t][:, 0:1], cval[:, 0, 0:1], 0.0,
                        scalar2=None, op0=ALU.mult)

            # transpose read -> readT
            ident = consts.tile([P, P], BF16, tag="ident", name="ident")
            make_identity(nc, ident[:])
            readT = [mem.tile([P, TOK], BF16, tag=f"mqT{c}", name=f"readT{c}") for c in range(ND)]
            for t in range(NT):
                for c in range(ND):
                    pt = psum_s.tile([P, P], BF16, tag="ps", name="ps")
                    nc.tensor.transpose(pt[:], read_bf[t][:, c * P:(c + 1) * P],
                                        ident[:])
                    nc.scalar.activation(readT[c][:, t * P:(t + 1) * P], pt[:],
                                         AF.Copy)

            # x += read @ w_read ; final rms norm
            wr_sb = [mem.tile([P, D], BF16, tag=f"wqm{c}", name=f"wr{c}") for c in range(ND)]
            for c in range(ND):
                nc.sync.dma_start(wr_sb[c][:], wr_d[c * P:(c + 1) * P, :])
            for m in range(ND):
                ps = psum_s.tile([P, TOK], F32, tag="ps", name="ps")
                for c in range(ND):
                    nc.tensor.matmul(ps[:], lhsT=wr_sb[c][:, m * P:(m + 1) * P],
                                     rhs=readT[c][:], start=(c == 0), stop=(c == ND - 1))
                nc.vector.tensor_add(xT[m][:], xT[m][:], ps[:])
                if flags["bias"]:
                    bt = load_bias_tile(msmall, br_d, m, "br")
                    nc.vector.tensor_scalar(xT[m][:], xT[m][:], bt[:, 0:1],
                                            scalar2=None, op0=ALU.add)
            rms_norm(xT, xoutq, no_d if flags["normw"] else None)
            if debug:
                for t in range(ND):
                    nc.sync.dma_start(dbg_xout[t * P:(t + 1) * P, :], xT[t][:])

        # ---------------- lm head (tied embeddings, vocab-sharded) ----------------
        if "lm" not in ablate:
         with (
            tc.tile_pool(name="et", bufs=3) as etp,
            tc.tile_pool(name="lmp", bufs=1) as lmp,
            tc.tile_pool(name="lout", bufs=4) as lout,
        ):
            NTT = (B * T) // P
            xo_in = dram.tile([D, TOK], BF16, tag="xo_in", name="xo_in")
            xo_out = dram.tile([N_CORES * D, TOK], BF16, tag="xo_out", name="xo_out")
            for m in range(ND):
                nc.sync.dma_start(xo_in[m * P:(m + 1) * P, :], xoutq[m][:])
            nc.gpsimd.collective_compute(
                "AllGather", ALU.bypass, replica_groups=[list(range(N_CORES))],
                ins=[xo_in.opt()], outs=[xo_out.opt()])
            xoo = xo_out[:].rearrange("(r x) c -> r x c", r=N_CORES)
            xall = [lmp.tile([P, B * T], BF16, tag=f"xall{c}", name=f"xall{c}")
                    for c in range(ND)]
            for c in range(ND):
                nc.sync.dma_start(
                    xall[c][:].rearrange("p (r t) -> p r t", r=N_CORES),
                    xoo[:, c * P:(c + 1) * P, :].rearrange("r p t -> p r t"))

            VC = 512
            nvc = (VSH + VC - 1) // VC
            for v in range(nvc):
                vn = min(VC, VSH - v * VC)
                et_sb = etp.tile([P, ND, VC], BF16, tag="et", name="et")
                nc.sync.dma_start(
                    et_sb[:, :, :vn],
                    eT_d[:, v * VC:v * VC + vn].rearrange("(c p) t -> p c t", p=P))
                for t in range(NTT):
                    ps = psum_s.tile([P, VC], F32, tag="ps", name="ps")
                    for c in range(ND):
                        nc.tensor.matmul(ps[:, :vn],
                                         lhsT=xall[c][:, t * P:(t + 1) * P],
                                         rhs=et_sb[:, c, :vn],
                                         start=(c == 0), stop=(c == ND - 1))
                    lo = lout.tile([P, VC], FP16, tag="lo", name="lo")
                    nc.scalar.activation(lo[:, :vn], ps[:, :vn], AF.Copy)
                    nc.sync.dma_start(
                        logits_d[t * P:(t + 1) * P, v * VC:v * VC + vn],
                        lo[:, :vn])

    nc.compile()
    return nc


# ---------------------------------------------------------------------------
# host-side sharding / assembly
# ---------------------------------------------------------------------------

def prep_inputs(inputs):
    bf = ml_dtypes.bfloat16
    ids = np.asarray(inputs["input_ids"])
    tok = np.asarray(inputs["tok_embed"], np.float32)
    pos = np.asarray(inputs["pos_embed"], np.float32)

    flags = {
        "bias": not (
            np.all(np.asarray(inputs["blk_bo"]) == 0)
            and np.all(np.asarray(inputs["blk_ffb1"]) == 0)
            and np.all(np.asarray(inputs["blk_ffb2"]) == 0)
            and np.all(np.asarray(inputs["bq_mem"]) == 0)
            and np.all(np.asarray(inputs["b_read"]) == 0)
        ),
        "normw": not (
            np.all(np.asarray(inputs["blk_norm1"]) == 1)
            and np.all(np.asarray(inputs["blk_norm2"]) == 1)
            and np.all(np.asarray(inputs["norm_out_w"]) == 1)
        ),
        "salience": not np.all(np.asarray(inputs["salience"]) == 0),
    }

    shared = {
        "wq": np.ascontiguousarray(np.asarray(inputs["blk_wq"]).astype(bf)),
        "wk": np.ascontiguousarray(np.asarray(inputs["blk_wk"]).astype(bf)),
        "wv": np.ascontiguousarray(np.asarray(inputs["blk_wv"]).astype(bf)),
        "wo": np.ascontiguousarray(np.asarray(inputs["blk_wo"]).astype(bf)),
        "w1": np.ascontiguousarray(np.asarray(inputs["blk_ffw1"]).astype(bf)),
        "w2": np.ascontiguousarray(np.asarray(inputs["blk_ffw2"]).astype(bf)),
        "wqm": np.ascontiguousarray(np.asarray(inputs["wq_mem"]).astype(bf)),
        "wr": np.ascontiguousarray(np.asarray(inputs["w_read"]).astype(bf)),
        "mv": np.ascontiguousarray(np.asarray(inputs["mem_V"]).astype(bf)),
    }
    mkT_full = np.asarray(inputs["mem_K"], np.float32).T.astype(bf)
    eT_full = tok.T.astype(bf)
    if flags["bias"]:
        shared.update(
            bo=np.asarray(inputs["blk_bo"], np.float32),
            b1=np.asarray(inputs["blk_ffb1"], np.float32),
            b2=np.asarray(inputs["blk_ffb2"], np.float32),
            bqm=np.asarray(inputs["bq_mem"], np.float32),
            br=np.asarray(inputs["b_read"], np.float32),
        )
    if flags["normw"]:
        shared.update(
            n1=np.asarray(inputs["blk_norm1"], np.float32),
            n2=np.asarray(inputs["blk_norm2"], np.float32),
            no=np.asarray(inputs["norm_out_w"], np.float32),
        )
    sal_full = np.asarray(inputs["salience"], np.float32)

    in_maps = []
    for c in range(N_CORES):
        b, p0 = c // 4, (c % 4) * TOK
        ids_c = ids[b, p0:p0 + TOK].astype(np.int64)
        m = dict(shared)
        m["x0T"] = np.ascontiguousarray(tok[ids_c].T)
        m["posT"] = np.ascontiguousarray(pos[p0:p0 + TOK].T)
        m["mkT"] = np.ascontiguousarray(mkT_full[:, c * SSH:(c + 1) * SSH])
        m["eT"] = np.ascontiguousarray(eT_full[:, c * VSH:(c + 1) * VSH])
        if flags["salience"]:
            m["sal"] = np.ascontiguousarray(sal_full[None, c * SSH:(c + 1) * SSH])
        in_maps.append(m)
    return in_maps, flags


def assemble(results):
    parts = [np.asarray(results[c]["logits"], np.float32) for c in range(N_CORES)]
    full = np.concatenate(parts, axis=1)        # [4096, 32000]
    return full.reshape(B, T, V_SIZE)


_PROGRAM_CACHE = {}


def get_program(flags):
    key = tuple(sorted(flags.items()))
    if key not in _PROGRAM_CACHE:
        _PROGRAM_CACHE[key] = build_program(flags)
    return _PROGRAM_CACHE[key]


def kernel(**inputs):
    in_maps, flags = prep_inputs(inputs)
    nc = get_program(flags)
    res = run_bass_kernel_spmd(nc, in_maps, list(range(N_CORES)))
    return assemble(res.results)



# revision 44
# speedup vs baseline: 2.3533x; 2.3533x over previous
"""Trainium2 Bass kernel for nn_BasicDNC (4-layer transformer + external
memory read + tied LM head), SPMD over 8 NeuronCores.

Sharding (v2):
  - tokens (B*T = 4096) split 512/core; cores 0-3 own batch 0, 4-7 batch 1
  - per-layer: AllGather the x activations (0.5MB in / 2MB out) within each
    4-core batch group; K/V for all 2048 group tokens recomputed locally
  - post-transformer is fully local (no collectives): each core scores its
    own 512 tokens against all 32768 memory slots (mem_K streamed from HBM
    as fp8 DoubleRow pairs), takes top-8 per token with DVE Max8/MaxIndex
    over 8192-slot quarters, gathers mem_V rows, and computes its own
    tokens' logits against the full vocab (tok_embed^T streamed). The LM
    head runs per tile-pair so it overlaps the later tiles' top-k scans.

Layouts: activations kept transposed ([d, tok]); scores / logits in
[tok, free] layout.
"""
import sys

sys.path.insert(0, "/opt/trn_rl_repo")

import numpy as np
import ml_dtypes

import concourse.bass as bass
import concourse.bacc as bacc
import concourse.mybir as mybir
import concourse.tile as tile
from concourse.bass_utils import run_bass_kernel_spmd
from concourse.masks import make_identity

F32 = mybir.dt.float32
BF16 = mybir.dt.bfloat16
FP16 = mybir.dt.float16
FP8 = mybir.dt.float8e4
U16 = mybir.dt.uint16
U32 = mybir.dt.uint32
I32 = mybir.dt.int32
AF = mybir.ActivationFunctionType
ALU = mybir.AluOpType
AX = mybir.AxisListType
PERF_DR = mybir.MatmulPerfMode.DoubleRow

N_CORES = 8
P = 128
D = 512
H = 8
DH = 64
L = 4
FF = 2048
B = 2
T = 2048
TOK = 512          # tokens per core
NT = TOK // P      # token tiles per core (4)
ND = D // P        # d tiles (4)
NFF = FF // P      # ff tiles (16)
NK = T // P        # key tiles per batch group (16)
GT = 4 * TOK       # tokens per batch group (2048)
SLOTS = 32768
QTR = 4096         # slots per scan unit
NQ = SLOTS // QTR  # 8
V_SIZE = 32000
TOPK = 8
EPS = 1e-8
VSW = 65           # per-head v-store width (64 v cols + 1 ones col)
FP8_SCALE = 64.0   # fp8 pre-scale for mq / mem_K (avoids e4m3 subnormals)
VC = 1000          # lm-head vocab chunk (psum pair)
NVC = V_SIZE // VC # 32
NCAND = NQ * TOPK  # 64 top-k candidates per token
SCORE_DR = True    # fp8 DoubleRow matmuls for memory scores (else plain fp8)
AG_FP8 = True      # fp8 payload for the per-layer x AllGather
GELU_AF = None     # override gelu activation (sim lacks Gelu_apprx_tanh)


def build_program(flags):
    nc = bacc.Bacc(None, num_devices=N_CORES)

    # ---------------- io ----------------
    x0T_d = nc.dram_tensor("x0T", [D, TOK], F32, kind="ExternalInput")
    posT_d = nc.dram_tensor("posT", [D, TOK], F32, kind="ExternalInput")
    wq_d = nc.dram_tensor("wq", [L, D, D], BF16, kind="ExternalInput")
    wk_d = nc.dram_tensor("wk", [L, D, D], BF16, kind="ExternalInput")
    wv_d = nc.dram_tensor("wv", [L, D, D], BF16, kind="ExternalInput")
    wo_d = nc.dram_tensor("wo", [L, D, D], BF16, kind="ExternalInput")
    w1_d = nc.dram_tensor("w1", [L, D, FF], BF16, kind="ExternalInput")
    w2_d = nc.dram_tensor("w2", [L, FF, D], BF16, kind="ExternalInput")
    wqm_d = nc.dram_tensor("wqm", [D, D], BF16, kind="ExternalInput")
    wr_d = nc.dram_tensor("wr", [D, D], BF16, kind="ExternalInput")
    mk8_d = nc.dram_tensor("mk8", [2, P, 2, SLOTS], FP8, kind="ExternalInput")
    mv_d = nc.dram_tensor("mv", [SLOTS, D], BF16, kind="ExternalInput")
    eT_d = nc.dram_tensor("eT", [D, V_SIZE], BF16, kind="ExternalInput")
    if flags["bias"]:
        bo_d = nc.dram_tensor("bo", [L, D], F32, kind="ExternalInput")
        b1_d = nc.dram_tensor("b1", [L, FF], F32, kind="ExternalInput")
        b2_d = nc.dram_tensor("b2", [L, D], F32, kind="ExternalInput")
        bqm_d = nc.dram_tensor("bqm", [D], F32, kind="ExternalInput")
        br_d = nc.dram_tensor("br", [D], F32, kind="ExternalInput")
    if flags["normw"]:
        n1_d = nc.dram_tensor("n1", [L, D], F32, kind="ExternalInput")
        n2_d = nc.dram_tensor("n2", [L, D], F32, kind="ExternalInput")
        no_d = nc.dram_tensor("no", [D], F32, kind="ExternalInput")
    if flags["salience"]:
        sal_d = nc.dram_tensor("sal", [1, SLOTS], F32, kind="ExternalInput")

    logits_d = nc.dram_tensor("logits", [TOK, V_SIZE], FP16, kind="ExternalOutput")

    groups = [[0, 1, 2, 3], [4, 5, 6, 7]]

    import contextlib

    with tile.TileContext(nc) as tc, contextlib.ExitStack() as ctx:
        persist = ctx.enter_context(tc.tile_pool(name="persist", bufs=1))
        consts = ctx.enter_context(tc.tile_pool(name="consts", bufs=1))
        dram = ctx.enter_context(tc.tile_pool(name="dram", bufs=2, space="DRAM"))
        small = ctx.enter_context(tc.tile_pool(name="small", bufs=2))

        ones_bf = consts.tile([P, 1], BF16, tag="ones_bf", name="ones_bf")
        nc.vector.memset(ones_bf[:], 1.0)
        eps_t = consts.tile([1, 1], F32, tag="eps_t", name="eps_t")
        nc.vector.memset(eps_t[:], EPS)

        # persistent residual stream (transposed): xT fp32 + bf16 shadow
        xT = [persist.tile([P, TOK], F32, tag=f"xT{t}", name=f"xT{t}") for t in range(ND)]
        xq = [persist.tile([P, TOK], BF16, tag=f"xq{t}", name=f"xq{t}") for t in range(ND)]

        def rms_norm(tiles_f32, tiles_bf16, normw_ap, psum_pool, c0=0, c1=TOK,
                     ssq_tag="ssq"):
            """In-place RMS norm over d of columns [c0:c1]; refresh bf16 shadow."""
            w = c1 - c0
            ssqt = psum_pool.tile([P, TOK], F32, tag=ssq_tag, name="ssqt")
            ssq = ssqt[0:1, :]
            for t in range(ND):
                sq = small.tile([P, TOK], BF16, tag="sq", name="sq")
                nc.vector.tensor_mul(sq[:, 0:w], tiles_f32[t][:, c0:c1],
                                     tiles_f32[t][:, c0:c1])
                nc.tensor.matmul(ssq[:, 0:w], lhsT=ones_bf[:], rhs=sq[:, 0:w],
                                 start=(t == 0), stop=(t == ND - 1))
            scale1 = small.tile([1, TOK], F32, tag="scale1", name="scale1")
            nc.scalar.activation(scale1[:, 0:w], ssq[:, 0:w], AF.Sqrt,
                                 bias=eps_t[:, 0:1], scale=1.0 / D)
            scaleb = small.tile([P, TOK], F32, tag="scaleb", name="scaleb")
            nc.gpsimd.partition_broadcast(scaleb[:, 0:w], scale1[:, 0:w])
            nc.vector.reciprocal(scaleb[:, 0:w], scaleb[:, 0:w])
            for t in range(ND):
                nc.vector.tensor_mul(tiles_f32[t][:, c0:c1], tiles_f32[t][:, c0:c1],
                                     scaleb[:, 0:w])
                if normw_ap is not None:
                    nw = small.tile([P, 1], F32, tag="nw", name="nw")
                    nc.sync.dma_start(nw[:], normw_ap[t * P:(t + 1) * P, None])
                    nc.vector.tensor_scalar(tiles_f32[t][:, c0:c1],
                                            tiles_f32[t][:, c0:c1],
                                            nw[:, 0:1], scalar2=None, op0=ALU.mult)
                nc.scalar.activation(tiles_bf16[t][:, c0:c1], tiles_f32[t][:, c0:c1],
                                     AF.Copy)

        def load_bias_tile(pool, ap_1d, t, tag):
            bt = pool.tile([P, 1], F32, tag=tag)
            nc.sync.dma_start(bt[:], ap_1d[t * P:(t + 1) * P, None])
            return bt

        # ---------------- embedding ----------------
        for t in range(ND):
            p0 = small.tile([P, TOK], F32, tag="emb0", name="emb0")
            p1 = small.tile([P, TOK], F32, tag="emb1", name="emb1")
            nc.sync.dma_start(p0[:], x0T_d[t * P:(t + 1) * P, :])
            nc.sync.dma_start(p1[:], posT_d[t * P:(t + 1) * P, :])
            nc.vector.tensor_add(xT[t][:], p0[:], p1[:])
            nc.scalar.activation(xq[t][:], xT[t][:], AF.Copy)

        # ---------------- transformer layers ----------------
        with (
            tc.tile_pool(name="wts", bufs=1) as wts,
            tc.tile_pool(name="attn", bufs=1) as attn,
            tc.tile_pool(name="psb", bufs=2) as psb,
            tc.tile_pool(name="ffh", bufs=1) as ffh,
            tc.tile_pool(name="psum_l", bufs=3, space="PSUM") as psum_l,
            tc.tile_pool(name="psum_a", bufs=1, space="PSUM") as psum_a,
        ):
            for l in range(L):
                # --- AllGather x within the 4-core batch group ---
                AGDT = FP8 if AG_FP8 else BF16
                ag_in = dram.tile([D, TOK], AGDT, tag="ag_in", name="ag_in")
                ag_out = dram.tile([4 * D, TOK], AGDT, tag="ag_out",
                                   name="ag_out")
                if AG_FP8:
                    x8 = attn.tile([P, ND, TOK], FP8, tag="x8", name="x8")
                    for m in range(ND):
                        nc.vector.tensor_copy(x8[:, m, :], xq[m][:])
                        nc.sync.dma_start(ag_in[m * P:(m + 1) * P, :], x8[:, m, :])
                else:
                    for m in range(ND):
                        nc.sync.dma_start(ag_in[m * P:(m + 1) * P, :], xq[m][:])
                nc.gpsimd.collective_compute(
                    "AllGather", ALU.bypass, replica_groups=groups,
                    ins=[ag_in.opt()], outs=[ag_out.opt()])

                # --- weights (DMA overlaps the AG) ---
                wq_sb = [wts.tile([P, D], BF16, tag=f"wq{c}", name=f"wq{c}") for c in range(ND)]
                wk_sb = [wts.tile([P, D], BF16, tag=f"wk{c}", name=f"wk{c}") for c in range(ND)]
                wv_sb = [wts.tile([P, D], BF16, tag=f"wv{c}", name=f"wv{c}") for c in range(ND)]
                wo2_sb = [wts.tile([P, D], BF16, tag=f"wo{c}", name=f"wo{c}") for c in range(ND)]
                for c in range(ND):
                    nc.sync.dma_start(wq_sb[c][:], wq_d[l, c * P:(c + 1) * P, :])
                    nc.sync.dma_start(wk_sb[c][:], wk_d[l, c * P:(c + 1) * P, :])
                    nc.sync.dma_start(wv_sb[c][:], wv_d[l, c * P:(c + 1) * P, :])
                    nc.sync.dma_start(wo2_sb[c][:], wo_d[l, c * P:(c + 1) * P, :])

                # --- Q projection from local xq (overlaps the AG) ---
                qT = [attn.tile([P, TOK], BF16, tag=f"qT{m}", name=f"qT{m}") for m in range(ND)]
                for mp in range(ND // 2):
                    ps2 = psum_l.tile([P, 2 * TOK], F32, tag="ps2", name="ps2")
                    for s in range(2):
                        m = mp * 2 + s
                        for c in range(ND):
                            nc.tensor.matmul(ps2[:, s * TOK:(s + 1) * TOK],
                                             lhsT=wq_sb[c][:, m * P:(m + 1) * P],
                                             rhs=xq[c][:], start=(c == 0),
                                             stop=(c == ND - 1))
                    for s in range(2):
                        nc.vector.tensor_copy(qT[mp * 2 + s][:],
                                              ps2[:, s * TOK:(s + 1) * TOK])

                # --- gathered x_all [d, 2048] ---
                ago = ag_out[:].rearrange("(r x) c -> r x c", r=4)
                xall = [attn.tile([P, GT], AGDT, tag=f"xall{c}", name=f"xall{c}")
                        for c in range(ND)]
                for c in range(ND):
                    nc.sync.dma_start(
                        xall[c][:].rearrange("p (r t) -> p r t", r=4),
                        ago[:, c * P:(c + 1) * P, :].rearrange("r p t -> p r t"))

                # --- K for all 2048 tokens: kfull[m] [128, 2048] ---
                kfull = [attn.tile([P, GT], BF16, tag=f"kfull{m}", name=f"kfull{m}")
                         for m in range(ND)]
                for m in range(ND):
                    for cp in range(2):
                        ps2 = psum_l.tile([P, 2 * TOK], F32, tag="ps2", name="ps2")
                        for s in range(2):
                            cc = cp * 2 + s
                            for c in range(ND):
                                nc.tensor.matmul(
                                    ps2[:, s * TOK:(s + 1) * TOK],
                                    lhsT=wk_sb[c][:, m * P:(m + 1) * P],
                                    rhs=xall[c][:, cc * TOK:(cc + 1) * TOK],
                                    start=(c == 0), stop=(c == ND - 1))
                        nc.vector.tensor_copy(
                            kfull[m][:, cp * 2 * TOK:(cp + 1) * 2 * TOK], ps2[:])

                # --- V for all tokens, packed per-head with ones column ---
                vst = [attn.tile([P, H, VSW], BF16, tag=f"vst{kt}", name=f"vst{kt}")
                       for kt in range(NK)]
                for kq in range(NK // 2):
                    ps2 = psum_l.tile([P, 2 * TOK], F32, tag="ps2", name="ps2")
                    for s in range(2):
                        kt = kq * 2 + s
                        for c in range(ND):
                            nc.tensor.matmul(ps2[:, s * TOK:(s + 1) * TOK],
                                             lhsT=xall[c][:, kt * P:(kt + 1) * P],
                                             rhs=wv_sb[c][:], start=(c == 0),
                                             stop=(c == ND - 1))
                    for s in range(2):
                        kt = kq * 2 + s
                        pv = ps2[:, s * TOK:(s + 1) * TOK].rearrange(
                            "p (h d) -> p h d", h=H)
                        nc.vector.tensor_copy(vst[kt][:, :, 0:DH], pv[:])
                        nc.vector.memset(vst[kt][:, :, DH:VSW], 1.0)

                # --- attention per head; ao packed in head pairs ---
                ao2 = [attn.tile([P, TOK], BF16, tag=f"ao2{c}", name=f"ao2{c}")
                       for c in range(ND)]
                for h in range(H):
                    dt_, lo = h // 2, (h % 2) * DH
                    pall = psb.tile([P, NK, TOK], BF16, tag="pexp", name="pexp")
                    for kp in range(NK // 2):
                        ps2 = psum_l.tile([P, 2 * TOK], F32, tag="ps2", name="ps2")
                        for s in range(2):
                            kt = kp * 2 + s
                            nc.tensor.matmul(
                                ps2[:, s * TOK:(s + 1) * TOK],
                                lhsT=kfull[dt_][lo:lo + DH, kt * P:(kt + 1) * P],
                                rhs=qT[dt_][lo:lo + DH, :], start=True, stop=True)
                        nc.scalar.activation(
                            pall[:, kp * 2:(kp + 1) * 2, :], ps2[:], AF.Exp,
                            scale=DH ** -0.5)
                    aops = psum_a.tile([P, TOK], F32, tag=f"ao{h % 2}", name=f"ao{h % 2}")
                    for kt in range(NK):
                        nc.tensor.matmul(aops[0:VSW, :], lhsT=vst[kt][:, h, :],
                                         rhs=pall[:, kt, :],
                                         start=(kt == 0), stop=(kt == NK - 1))
                    # denominator: row DH -> partition bcast -> recip -> normalize
                    den1 = attn.tile([P, TOK], F32, tag="den1", name="den1")
                    nc.vector.tensor_copy(den1[DH:DH + 1, :], aops[DH:DH + 1, :])
                    den0 = attn.tile([1, TOK], F32, tag="den0", name="den0")
                    nc.sync.dma_start(den0[:], den1[DH:DH + 1, :])
                    denb = attn.tile([P, TOK], F32, tag="denb", name="denb")
                    nc.gpsimd.partition_broadcast(denb[:], den0[:])
                    nc.vector.reciprocal(denb[0:DH, :], denb[0:DH, :])
                    nc.vector.tensor_mul(ao2[h // 2][lo:lo + DH, :],
                                         aops[0:DH, :], denb[0:DH, :])

                # --- output projection (head pairs) + residual + norm1 ---
                for mp in range(ND // 2):
                    ps2 = psum_l.tile([P, 2 * TOK], F32, tag="ps2", name="ps2")
                    for s in range(2):
                        m = mp * 2 + s
                        for c in range(ND):
                            nc.tensor.matmul(
                                ps2[:, s * TOK:(s + 1) * TOK],
                                lhsT=wo2_sb[c][:, m * P:(m + 1) * P],
                                rhs=ao2[c][:], start=(c == 0), stop=(c == ND - 1))
                    for s in range(2):
                        m = mp * 2 + s
                        nc.vector.tensor_add(xT[m][:], xT[m][:],
                                             ps2[:, s * TOK:(s + 1) * TOK])
                        if flags["bias"]:
                            bt = load_bias_tile(small, bo_d[l], m, "bo")
                            nc.vector.tensor_scalar(xT[m][:], xT[m][:], bt[:, 0:1],
                                                    scalar2=None, op0=ALU.add)
                rms_norm(xT, xq, n1_d[l] if flags["normw"] else None, psum_a,
                         ssq_tag="ao0")

                # --- FFN + residual + norm2 ---
                w1_sb = [wts.tile([P, FF], BF16, tag=f"w1_{c}", name=f"w1_{c}") for c in range(ND)]
                w2_sb = [wts.tile([P, D], BF16, tag=f"w2_{c}", name=f"w2_{c}") for c in range(NFF)]
                for c in range(ND):
                    nc.sync.dma_start(w1_sb[c][:], w1_d[l, c * P:(c + 1) * P, :])
                for c in range(NFF):
                    nc.sync.dma_start(w2_sb[c][:], w2_d[l, c * P:(c + 1) * P, :])
                hT2 = ffh.tile([P, NFF, TOK], BF16, tag="hT2", name="hT2")
                for fp_ in range(NFF // 2):
                    ps2 = psum_l.tile([P, 2 * TOK], F32, tag="ps2", name="ps2")
                    for s in range(2):
                        f = fp_ * 2 + s
                        for c in range(ND):
                            nc.tensor.matmul(ps2[:, s * TOK:(s + 1) * TOK],
                                             lhsT=w1_sb[c][:, f * P:(f + 1) * P],
                                             rhs=xq[c][:], start=(c == 0),
                                             stop=(c == ND - 1))
                    if flags["bias"]:
                        for s in range(2):
                            f = fp_ * 2 + s
                            bt = load_bias_tile(small, b1_d[l], f, "b1")
                            nc.scalar.activation(hT2[:, f, :],
                                                 ps2[:, s * TOK:(s + 1) * TOK],
                                                 AF.Gelu_apprx_tanh, bias=bt[:, 0:1])
                    elif GELU_AF == "sig":
                        # sim-only: gelu(x) ~= x * sigmoid(1.702 x)
                        sg = small.tile([P, 2 * TOK], BF16, tag="sg", name="sg")
                        nc.scalar.activation(sg[:], ps2[:], AF.Sigmoid, scale=1.702)
                        nc.vector.tensor_tensor(
                            out=hT2[:, fp_ * 2:(fp_ + 1) * 2, :].rearrange(
                                "p a b -> p (a b)"),
                            in0=ps2[:], in1=sg[:], op=ALU.mult)
                    else:
                        nc.scalar.activation(hT2[:, fp_ * 2:(fp_ + 1) * 2, :],
                                             ps2[:], AF.Gelu_apprx_tanh)
                for mp in range(ND // 2):
                    ps2 = psum_l.tile([P, 2 * TOK], F32, tag="ps2", name="ps2")
                    for s in range(2):
                        m = mp * 2 + s
                        for c in range(NFF):
                            nc.tensor.matmul(ps2[:, s * TOK:(s + 1) * TOK],
                                             lhsT=w2_sb[c][:, m * P:(m + 1) * P],
                                             rhs=hT2[:, c, :], start=(c == 0),
                                             stop=(c == NFF - 1))
                    for s in range(2):
                        m = mp * 2 + s
                        nc.vector.tensor_add(xT[m][:], xT[m][:],
                                             ps2[:, s * TOK:(s + 1) * TOK])
                        if flags["bias"]:
                            bt = load_bias_tile(small, b2_d[l], m, "b2")
                            nc.vector.tensor_scalar(xT[m][:], xT[m][:], bt[:, 0:1],
                                                    scalar2=None, op0=ALU.add)
                rms_norm(xT, xq, n2_d[l] if flags["normw"] else None, psum_a,
                         ssq_tag="ao0")

        # ---------------- external memory read + lm head (fully local) -------
        xoutq = [persist.tile([P, TOK], BF16, tag=f"xoq{t}", name=f"xoq{t}") for t in range(ND)]
        with (
            tc.tile_pool(name="mem", bufs=1) as mem,
            tc.tile_pool(name="scpool", bufs=8) as scpool,
            tc.tile_pool(name="mkpool", bufs=2) as mkpool,
            tc.tile_pool(name="msmall", bufs=2) as msmall,
            tc.tile_pool(name="gat", bufs=1) as gat,
            tc.tile_pool(name="etp", bufs=2) as etp,
            tc.tile_pool(name="loutp", bufs=2) as loutp,
            tc.tile_pool(name="psum_pp", bufs=2, space="PSUM") as psum_pp,
            tc.tile_pool(name="psum_m", bufs=1, space="PSUM") as psum_m,
        ):
            # mq^T = wqm^T x, written as fp8 DoubleRow pairs (scaled)
            wqm_sb = [mem.tile([P, D], BF16, tag=f"wqm{c}", name=f"wqm{c}") for c in range(ND)]
            for c in range(ND):
                nc.sync.dma_start(wqm_sb[c][:], wqm_d[c * P:(c + 1) * P, :])
            mq8 = [mem.tile([P, 2, TOK], FP8, tag=f"mq8{g}", name=f"mq8{g}")
                   for g in range(2)]
            sc_mq = float(D ** -0.5) * FP8_SCALE
            for m in range(ND):
                g, i = m // 2, m % 2
                ps = psum_m.tile([P, TOK], F32, tag="ps", name="ps")
                for c in range(ND):
                    nc.tensor.matmul(ps[:], lhsT=wqm_sb[c][:, m * P:(m + 1) * P],
                                     rhs=xq[c][:], start=(c == 0), stop=(c == ND - 1))
                if flags["bias"]:
                    bt = load_bias_tile(msmall, bqm_d, m, "bqm")
                    sc1 = msmall.tile([P, 1], F32, tag="bqms", name="bqms")
                    nc.vector.tensor_scalar(sc1[:], bt[:], sc_mq,
                                            scalar2=None, op0=ALU.mult)
                    nc.scalar.activation(mq8[g][:, i, :], ps[:], AF.Identity,
                                         bias=sc1[:, 0:1], scale=sc_mq)
                else:
                    nc.scalar.activation(mq8[g][:, i, :], ps[:], AF.Copy,
                                         scale=sc_mq)

            if flags["salience"]:
                salq = [mem.tile([P, QTR], FP16, tag=f"salq{q}", name=f"salq{q}")
                        for q in range(NQ)]
                sal1 = mem.tile([1, SLOTS], F32, tag="sal1", name="sal1")
                nc.sync.dma_start(sal1[:], sal_d[:])
                for q in range(NQ):
                    nc.gpsimd.partition_broadcast(
                        salq[q][:], sal1[:, q * QTR:(q + 1) * QTR])

            iota32i = consts.tile([P, NCAND], I32, tag="iota_i", name="iota_i")
            nc.gpsimd.iota(iota32i[:], pattern=[[1, NCAND]], base=0,
                           channel_multiplier=0)
            iota32f = consts.tile([P, NCAND], F32, tag="iota_f", name="iota_f")
            nc.vector.tensor_copy(iota32f[:], iota32i[:])

            ident = consts.tile([P, P], BF16, tag="ident", name="ident")
            make_identity(nc, ident[:])
            wr_sb = [mem.tile([P, D], BF16, tag=f"wr{c}", name=f"wr{c}") for c in range(ND)]
            for c in range(ND):
                nc.sync.dma_start(wr_sb[c][:], wr_d[c * P:(c + 1) * P, :])
            readT = [mem.tile([P, TOK], BF16, tag=f"readT{c}", name=f"readT{c}")
                     for c in range(ND)]

            # per-(tile, quarter) top-8 candidates
            hv = [mem.tile([P, NCAND], FP16, tag=f"hv{t}", name=f"hv{t}") for t in range(NT)]
            hi = [mem.tile([P, NCAND], U16, tag=f"hi{t}", name=f"hi{t}") for t in range(NT)]
            read_bf = [mem.tile([P, D], BF16, tag=f"read{t}", name=f"read{t}") for t in range(NT)]
            idxu_t = [mem.tile([P, TOPK], U32, tag=f"idxu{t}", name=f"idxu{t}") for t in range(NT)]
            w8_t = [mem.tile([P, TOPK], F32, tag=f"w8{t}", name=f"w8{t}") for t in range(NT)]

            sct_scale = float(1.0 / (FP8_SCALE * FP8_SCALE))

            def topk_merge(t):
                """Merge quarter top-8s into global top-8 + softmax weights."""
                cand_if = msmall.tile([P, NCAND], F32, tag="candif", name="candif")
                nc.vector.tensor_copy(cand_if[:], hi[t][:])
                for q in range(1, NQ):
                    nc.vector.tensor_scalar(
                        cand_if[:, q * 8:(q + 1) * 8], cand_if[:, q * 8:(q + 1) * 8],
                        float(q * QTR), scalar2=None, op0=ALU.add)
                v8 = msmall.tile([P, 8], FP16, tag="v8", name="v8")
                nc.vector.max(out=v8[:], in_=hv[t][:])
                pos = msmall.tile([P, 8], U16, tag="pos", name="pos")
                nc.vector.max_index(out=pos[:], in_max=v8[:], in_values=hv[t][:])
                posf = msmall.tile([P, 8], F32, tag="posf", name="posf")
                nc.vector.tensor_copy(posf[:], pos[:])
                eq = msmall.tile([P, 8, NCAND], F32, tag="eq", name="eq")
                nc.vector.tensor_tensor(
                    out=eq[:], in0=posf[:, :, None].to_broadcast([P, 8, NCAND]),
                    in1=iota32f[:, None, :].to_broadcast([P, 8, NCAND]),
                    op=ALU.is_equal)
                nc.vector.tensor_tensor(
                    out=eq[:], in0=eq[:],
                    in1=cand_if[:, None, :].to_broadcast([P, 8, NCAND]),
                    op=ALU.mult)
                idxf = msmall.tile([P, 8], F32, tag="idxf", name="idxf")
                nc.vector.reduce_sum(idxf[:], eq[:], axis=AX.X)
                nc.vector.tensor_copy(idxu_t[t][:], idxf[:])
                # softmax over the 8 scores
                vf = msmall.tile([P, 8], F32, tag="vf", name="vf")
                nc.vector.tensor_copy(vf[:], v8[:])
                ew = msmall.tile([P, 8], F32, tag="ew", name="ew")
                nc.scalar.activation(ew[:], vf[:], AF.Exp)
                ssum = msmall.tile([P, 1], F32, tag="ssum", name="ssum")
                nc.vector.reduce_sum(ssum[:], ew[:], axis=AX.X)
                rs = msmall.tile([P, 1], F32, tag="rs", name="rs")
                nc.vector.reciprocal(rs[:], ssum[:])
                nc.vector.tensor_scalar(w8_t[t][:], ew[:], rs[:, 0:1],
                                        scalar2=None, op0=ALU.mult)

            vsel_t = {}

            def mem_gather(t):
                vsel = gat.tile([P, TOPK, D], BF16, tag=f"vsel{t % 2}",
                                name=f"vsel{t % 2}")
                vsel_t[t] = vsel
                for k in range(TOPK):
                    nc.gpsimd.indirect_dma_start(
                        out=vsel[:, k, :], out_offset=None, in_=mv_d[:],
                        in_offset=bass.IndirectOffsetOnAxis(
                            ap=idxu_t[t][:, k:k + 1], axis=0),
                        bounds_check=SLOTS - 1, oob_is_err=False)

            def mem_read(t):
                vsel = vsel_t[t]
                racc = msmall.tile([P, D], F32, tag="racc", name="racc")
                rtmp = msmall.tile([P, D], F32, tag="rtmp", name="rtmp")
                nc.vector.tensor_scalar(racc[:], vsel[:, 0, :], w8_t[t][:, 0:1],
                                        scalar2=None, op0=ALU.mult)
                for k in range(1, TOPK):
                    nc.vector.tensor_scalar(rtmp[:], vsel[:, k, :], w8_t[t][:, k:k + 1],
                                            scalar2=None, op0=ALU.mult)
                    nc.vector.tensor_add(racc[:], racc[:], rtmp[:])
                nc.vector.tensor_copy(read_bf[t][:], racc[:])

            def topk_reads_pair(pair):
                """top-k merge + mem_V gather + weighted read for the pair's
                tiles (DVE/Pool only — no PE). Gathers issued for both tiles
                before the weighted sums so the DMA hides behind DVE work."""
                t0 = pair * 2
                for t in (t0, t0 + 1):
                    topk_merge(t)
                for t in (t0, t0 + 1):
                    mem_gather(t)
                for t in (t0, t0 + 1):
                    mem_read(t)

            def read_tail_pair(pair):
                """read transpose + read proj + final norm -> xoutq columns."""
                t0 = pair * 2
                # transpose read -> readT columns
                for t in (t0, t0 + 1):
                    for c in range(ND):
                        pt = psum_m.tile([P, P], BF16, tag="pst", name="pst")
                        nc.tensor.transpose(pt[:], read_bf[t][:, c * P:(c + 1) * P],
                                            ident[:])
                        nc.scalar.activation(readT[c][:, t * P:(t + 1) * P], pt[:],
                                             AF.Copy)
                # x += read @ w_read on this pair's columns; final rms norm
                c0, c1 = t0 * P, (t0 + 2) * P
                for m in range(ND):
                    ps = psum_m.tile([P, TOK], F32, tag="ps", name="ps")
                    for c in range(ND):
                        nc.tensor.matmul(ps[:, 0:c1 - c0],
                                         lhsT=wr_sb[c][:, m * P:(m + 1) * P],
                                         rhs=readT[c][:, c0:c1],
                                         start=(c == 0), stop=(c == ND - 1))
                    nc.vector.tensor_add(xT[m][:, c0:c1], xT[m][:, c0:c1],
                                         ps[:, 0:c1 - c0])
                    if flags["bias"]:
                        bt = load_bias_tile(msmall, br_d, m, "br")
                        nc.vector.tensor_scalar(xT[m][:, c0:c1], xT[m][:, c0:c1],
                                                bt[:, 0:1], scalar2=None, op0=ALU.add)
                rms_norm(xT, xoutq, no_d if flags["normw"] else None, psum_m,
                         c0=c0, c1=c1)

            def lm_chunks(pair, v0, v1, tail=False):
                """lm head vocab chunks [v0, v1) for tiles {2*pair, 2*pair+1}.
                tail=True issues logits stores from DVE (idle after scans)."""
                t0 = pair * 2
                st_eng = nc.sync
                for v in range(v0, v1):
                    off = v * VC
                    et_sb = etp.tile([P, ND, VC], BF16, tag="et", name="et")
                    eng = nc.sync if v % 2 == 0 else nc.gpsimd
                    eng.dma_start(
                        et_sb[:],
                        eT_d[:, off:off + VC].rearrange("(c p) v -> p c v", p=P))
                    lo = loutp.tile([P, 2, VC], FP16, tag="lo", name="lo")
                    for ti in range(2):
                        t = t0 + ti
                        ps = psum_pp.tile([P, 1024], F32, tag="pp", name="pp")
                        # 500-wide halves placed at 512-aligned offsets so each
                        # matmul stays within one psum bank
                        for sub in range(2):
                            for c in range(ND):
                                nc.tensor.matmul(
                                    ps[:, sub * 512:sub * 512 + 500],
                                    lhsT=xoutq[c][:, t * P:(t + 1) * P],
                                    rhs=et_sb[:, c, sub * 500:(sub + 1) * 500],
                                    start=(c == 0), stop=(c == ND - 1))
                        ps_v = ps[:].rearrange("p (a b) -> p a b", a=2)[:, :, 0:500]
                        lo_v = lo[:, ti, :].rearrange("p (a b) -> p a b", a=2)
                        nc.scalar.activation(lo_v, ps_v, AF.Copy)
                    st_eng.dma_start(
                        logits_d[t0 * P:(t0 + 2) * P, off:off + VC].rearrange(
                            "(t p) v -> p t v", p=P),
                        lo[:])

            def score_quarter(pair, q):
                """score the pair's two tiles against slot quarter q; per-unit
                top-8 candidates into hv/hi."""
                NPPQ = QTR // 1024  # psum pairs per (tile, unit)
                mkq = [mkpool.tile([P, 2, QTR], FP8, tag=f"mkq{g}",
                                   name=f"mkq{g}") for g in range(2)]
                for g in range(2):
                    nc.sync.dma_start(mkq[g][:],
                                      mk8_d[g, :, :, q * QTR:(q + 1) * QTR])
                for ti in range(2):
                    t = pair * 2 + ti
                    sct = scpool.tile([P, QTR], FP16, tag="sct", name="sct")
                    for pp in range(NPPQ):
                        ps = psum_pp.tile([P, 1024], F32, tag="pp", name="pp")
                        for sub in range(2):
                            lo_, hi_ = (pp * 1024 + sub * 512,
                                        pp * 1024 + (sub + 1) * 512)
                            if SCORE_DR:
                                for g in range(2):
                                    nc.tensor.matmul(
                                        ps[:, sub * 512:(sub + 1) * 512],
                                        lhsT=mq8[g][:, :, t * P:(t + 1) * P],
                                        rhs=mkq[g][:, :, lo_:hi_],
                                        start=(g == 0), stop=(g == 1),
                                        perf_mode=PERF_DR)
                            else:
                                for g in range(2):
                                    for i in range(2):
                                        nc.tensor.matmul(
                                            ps[:, sub * 512:(sub + 1) * 512],
                                            lhsT=mq8[g][:, i, t * P:(t + 1) * P],
                                            rhs=mkq[g][:, i, lo_:hi_],
                                            start=(g == 0 and i == 0),
                                            stop=(g == 1 and i == 1))
                        nc.scalar.activation(sct[:, pp * 1024:(pp + 1) * 1024],
                                             ps[:], AF.Copy, scale=sct_scale)
                    if flags["salience"]:
                        nc.vector.tensor_add(sct[:], sct[:], salq[q][:])
                    nc.vector.max(out=hv[t][:, q * 8:q * 8 + 8], in_=sct[:])
                    nc.vector.max_index(out=hi[t][:, q * 8:q * 8 + 8],
                                        in_max=hv[t][:, q * 8:q * 8 + 8],
                                        in_values=sct[:])

            # Emission order is per-engine execution order (in-order engines).
            # Weave pair-1 scores and the pair-0 lm head so that DVE scans
            # continuously while PE alternates score matmuls and lm chunks.
            for q in range(NQ):
                score_quarter(0, q)
            topk_reads_pair(0)
            read_tail_pair(0)
            for q in range(NQ // 2):
                score_quarter(1, q)
            for j in range(NQ // 2):
                score_quarter(1, NQ // 2 + j)
                lm_chunks(0, j * 4, (j + 1) * 4)
            topk_reads_pair(1)
            lm_chunks(0, NQ // 2 * 4, NVC, tail=True)
            read_tail_pair(1)
            lm_chunks(1, 0, NVC, tail=True)

    nc.compile()
    return nc


# ---------------------------------------------------------------------------
# host-side sharding / assembly
# ---------------------------------------------------------------------------

def prep_inputs(inputs):
    bf = ml_dtypes.bfloat16
    f8 = ml_dtypes.float8_e4m3fn
    ids = np.asarray(inputs["input_ids"])
    tok = np.asarray(inputs["tok_embed"], np.float32)
    pos = np.asarray(inputs["pos_embed"], np.float32)

    flags = {
        "bias": not (
            np.all(np.asarray(inputs["blk_bo"]) == 0)
            and np.all(np.asarray(inputs["blk_ffb1"]) == 0)
            and np.all(np.asarray(inputs["blk_ffb2"]) == 0)
            and np.all(np.asarray(inputs["bq_mem"]) == 0)
            and np.all(np.asarray(inputs["b_read"]) == 0)
        ),
        "normw": not (
            np.all(np.asarray(inputs["blk_norm1"]) == 1)
            and np.all(np.asarray(inputs["blk_norm2"]) == 1)
            and np.all(np.asarray(inputs["norm_out_w"]) == 1)
        ),
        "salience": not np.all(np.asarray(inputs["salience"]) == 0),
    }

    # mem_K packed for fp8 DoubleRow: mk8[g, p, i, s] = K[s, g*256+i*128+p]*S
    mkT = np.asarray(inputs["mem_K"], np.float32).T * FP8_SCALE  # [D, S]
    mk8 = np.ascontiguousarray(
        mkT.reshape(2, 2, P, SLOTS).transpose(0, 2, 1, 3).astype(f8))

    shared = {
        "wq": np.ascontiguousarray(np.asarray(inputs["blk_wq"]).astype(bf)),
        "wk": np.ascontiguousarray(np.asarray(inputs["blk_wk"]).astype(bf)),
        "wv": np.ascontiguousarray(np.asarray(inputs["blk_wv"]).astype(bf)),
        "wo": np.ascontiguousarray(np.asarray(inputs["blk_wo"]).astype(bf)),
        "w1": np.ascontiguousarray(np.asarray(inputs["blk_ffw1"]).astype(bf)),
        "w2": np.ascontiguousarray(np.asarray(inputs["blk_ffw2"]).astype(bf)),
        "wqm": np.ascontiguousarray(np.asarray(inputs["wq_mem"]).astype(bf)),
        "wr": np.ascontiguousarray(np.asarray(inputs["w_read"]).astype(bf)),
        "mv": np.ascontiguousarray(np.asarray(inputs["mem_V"]).astype(bf)),
        "mk8": mk8,
        "eT": np.ascontiguousarray(tok.T.astype(bf)),
    }
    if flags["bias"]:
        shared.update(
            bo=np.asarray(inputs["blk_bo"], np.float32),
            b1=np.asarray(inputs["blk_ffb1"], np.float32),
            b2=np.asarray(inputs["blk_ffb2"], np.float32),
            bqm=np.asarray(inputs["bq_mem"], np.float32),
            br=np.asarray(inputs["b_read"], np.float32),
        )
    if flags["normw"]:
        shared.update(
            n1=np.asarray(inputs["blk_norm1"], np.float32),
            n2=np.asarray(inputs["blk_norm2"], np.float32),
            no=np.asarray(inputs["norm_out_w"], np.float32),
        )
    if flags["salience"]:
        shared["sal"] = np.ascontiguousarray(
            np.asarray(inputs["salience"], np.float32)[None, :])

    in_maps = []
    for c in range(N_CORES):
        b, p0 = c // 4, (c % 4) * TOK
        ids_c = ids[b, p0:p0 + TOK].astype(np.int64)
        m = dict(shared)
        m["x0T"] = np.ascontiguousarray(tok[ids_c].T)
        m["posT"] = np.ascontiguousarray(pos[p0:p0 + TOK].T)
        in_maps.append(m)
    return in_maps, flags


def assemble(results):
    parts = [np.asarray(results[c]["logits"], np.float32) for c in range(N_CORES)]
    full = np.concatenate(parts, axis=0)          # [4096, 32000]
    return full.reshape(B, T, V_SIZE)


_PROGRAM_CACHE = {}


def get_program(flags):
    key = tuple(sorted(flags.items()))
    if key not in _PROGRAM_CACHE:
        _PROGRAM_CACHE[key] = build_program(flags)
    return _PROGRAM_CACHE[key]


def kernel(**inputs):
    in_maps, flags = prep_inputs(inputs)
    nc = get_program(flags)
    res = run_bass_kernel_spmd(nc, in_maps, list(range(N_CORES)))
    return assemble(res.results)
